# revision 19
# baseline (speedup 1.0000x reference)
"""Trainium2 Bass kernel for nn_FFN_pairwise_z (pairwise-concat FFN scoring).

Math (see reference):
    a = op @ W1[:z]           [N_op, h]
    b = co @ W1[z:]           [N_co, h]
    score_ij = relu( relu(a_i + b_j + b1) . W2 + b2 )
    OP_w[i] = sum_j score, CO_w[j] = sum_i score, T = sum_ij score
    out = (OP_w @ op / T,  CO_w @ co / T)       two [1, z] vectors

Sharding: N_op rows split across 8 cores (128 rows each).  Each core
computes its score block [128, 1024] without materializing it in DRAM and
emits only partial sums:
    u_op_part   = OP_w_local @ op_local        [z]
    T_part      = sum(OP_w_local)              [1]
    u_co_part   = CO_w_part @ co               [z]
packed as one [1, 2z+1] output.  The host adds the 8 partials and divides
by T (the "all-reduce + normalize" step of the hinted strategy, done on
host since it is 257 floats).

Device pipeline per core (layout: h on partitions):
    bT   = (co @ W1b)^T     [h=128, N_co]   fp16, via 2 fp32 matmuls
    abias= (op_l @ W1a)^T + b1  [h, 128]    fp32
    per i in 0..127:
        hid_i = max(bT + abias[:, i], 0)    one DVE tensor_scalar (fp16, 4x)
        s[i, :] = W2^T @ hid_i              two fp16 matmuls -> PSUM row i
    score = relu(s + b2) (ACT, accum_out gives OP_w_local for free)
    u_op|T  : one matmul  lhsT=OP_w_local, rhs=[op_l | ones]
    CO_w^T  : 8 matmuls   lhsT=score chunk, rhs=ones
    u_co    : 8 accumulating matmuls lhsT=CO_w^T col, rhs=co chunk
"""

import os
import sys

for _p in ("/opt/trn_rl_repo", "/root/.axon_site/_ro/trn_rl_repo"):
    if os.path.isdir(_p) and _p not in sys.path:
        sys.path.insert(0, _p)

import numpy as np

import concourse.bacc as bacc
import concourse.tile as tile
from concourse import mybir
from concourse.bass_utils import run_bass_kernel_spmd

N_OP, N_CO, Z, H = 1024, 1024, 128, 128
N_CORES = 8
ROWS = N_OP // N_CORES  # 128 op-rows per core
F32 = mybir.dt.float32
F16 = mybir.dt.float16
OUT_W = 2 * Z + 1  # u_op (z) | T (1) | u_co (z)

_CACHE = {}
LAST_EXEC_NS = None


def _build():
    nc = bacc.Bacc("TRN2", target_bir_lowering=False, debug=False)

    op_ext = nc.dram_tensor("op_ext", [ROWS, Z + 1], F32, kind="ExternalInput")
    coT = nc.dram_tensor("coT", [Z, N_CO], F16, kind="ExternalInput")
    co_pk = nc.dram_tensor("co_pk", [128, N_CO], F16, kind="ExternalInput")
    # w1b | w1a | op_lT packed as one fp16 tensor (single DMA)
    wpack = nc.dram_tensor("wpack", [Z, 2 * H + ROWS], F16, kind="ExternalInput")
    # single row: [b1 (128) | W2 (128) | b2 (1)] fp16 (single DMA)
    vpack = nc.dram_tensor("vpack", [1, 2 * H + 1], F16, kind="ExternalInput")
    out = nc.dram_tensor("out", [1, OUT_W], F32, kind="ExternalOutput")

    with tile.TileContext(nc) as tc:
        with (
            tc.tile_pool(name="singles", bufs=1) as singles,
            tc.tile_pool(name="hidp", bufs=4) as hidp,
            tc.tile_pool(name="ps_main", bufs=1, space="PSUM") as psm,
            tc.tile_pool(name="ps_tmp", bufs=2, space="PSUM") as pst,
        ):
            # 5 input DMAs total, issue spread across three engines so the
            # per-dma_start descriptor-gen cost (~0.6us) does not serialize.
            sb_coT = singles.tile([128, N_CO], F16)
            nc.sync.dma_start(out=sb_coT[:, 0:512], in_=coT[:, 0:512])
            nc.scalar.dma_start(out=sb_coT[:, 512:1024], in_=coT[:, 512:1024])
            sb_wpack = singles.tile([128, 2 * H + ROWS], F16)
            nc.gpsimd.dma_start(out=sb_wpack[:, :], in_=wpack[:, :])
            sb_w1b = sb_wpack[:, 0:H]
            sb_w1a = sb_wpack[:, H : 2 * H]
            sb_oplT = sb_wpack[:, 2 * H : 2 * H + ROWS]
            sb_vpack = singles.tile([1, 2 * H + 1], F16)
            nc.scalar.dma_start(out=sb_vpack[0:1, :], in_=vpack[0:1, :])
            sb_b1r = sb_vpack[0:1, 0:H]
            sb_w2r = sb_vpack[0:1, H : 2 * H]
            sb_b2cell = sb_vpack[0:1, 2 * H : 2 * H + 1]
            # late-needed loads (tail only)
            sb_copk = singles.tile([128, N_CO], F16)
            nc.gpsimd.dma_start(out=sb_copk[:, :], in_=co_pk[:, :])
            sb_opext = singles.tile([128, Z + 1], F32)
            nc.scalar.dma_start(out=sb_opext[:, :], in_=op_ext[:, :])

            # on-chip constants / broadcasts (no partition-scattered DMAs)
            sb_onesrow = singles.tile([1, ROWS], F16)
            nc.vector.memset(sb_onesrow[0:1, :], 1.0)
            sb_ident = singles.tile([1, 1], F16)
            nc.vector.memset(sb_ident[0:1, :], 1.0)
            sb_one = singles.tile([128, 1], F32)
            nc.vector.memset(sb_one[:, :], 1.0)

            # w2 column via PE transpose of the [1,128] row
            ps_w2 = pst.tile([128, 1], F16, tag="tmp")
            nc.tensor.transpose(ps_w2[:, :], sb_w2r[0:1, :], sb_ident[0:1, :])
            sb_w2 = singles.tile([128, 1], F16)
            nc.vector.tensor_copy(sb_w2[:, :], ps_w2[:, :])

            # b2 column: [128,1] broadcast of the scalar via K=1 matmul
            ps_b2 = pst.tile([128, 1], F32, tag="tmp")
            nc.tensor.matmul(ps_b2[:, :], lhsT=sb_onesrow[0:1, :], rhs=sb_b2cell[0:1, :], start=True, stop=True)
            sb_b2 = singles.tile([128, 1], F32)
            nc.vector.tensor_copy(sb_b2[:, :], ps_b2[:, :])

            # abias[h, i] = sum_z W1a[z,h] opT[z,i] + b1[h] (b1 folded via K=1
            # accumulate matmul: lhsT=b1row, rhs=ones_row)
            ps_a = pst.tile([128, ROWS], F32, tag="tmp")
            nc.tensor.matmul(ps_a[:, :], lhsT=sb_w1a[:, :], rhs=sb_oplT[:, :], start=True, stop=False)
            nc.tensor.matmul(ps_a[:, :], lhsT=sb_b1r[0:1, :], rhs=sb_onesrow[0:1, :], start=False, stop=True)
            sb_abias = singles.tile([128, ROWS], F32)
            nc.vector.tensor_copy(sb_abias[:, :], ps_a[:, :])

            # bT[h, j] = sum_z W1b[z, h] * coT[z, j], stored fp16
            sb_bT = singles.tile([128, N_CO], F16)
            for half in range(2):
                ps_b = pst.tile([128, 512], F32, tag="tmp")
                nc.tensor.matmul(
                    ps_b[:, :],
                    lhsT=sb_w1b[:, :],
                    rhs=sb_coT[:, half * 512 : (half + 1) * 512],
                    start=True,
                    stop=True,
                )
                nc.scalar.copy(sb_bT[:, half * 512 : (half + 1) * 512], ps_b)

            # main pairwise loop.  hid chunks go through the PE as the
            # STATIONARY operand (fp16 weight loads stream 2 elem/cycle),
            # W2 as the moving operand (N=1): one [128,1] psum column per
            # (i, j-chunk), written at free offset (c%4)*128 + i.
            # ps_t0 holds j-chunks 0..3, ps_t1 chunks 4..7; layout [j, (c, i)].
            ps_t0 = psm.tile([128, 512], F32, tag="s0")
            ps_t1 = psm.tile([128, 512], F32, tag="s1")
            ps_t = (ps_t0, ps_t1)
            WARM = 16

            # warm-up phase: first WARM rows in half-tiles so PE can start
            # on bT half 0 while bT half 1 is still being produced
            for i in range(WARM):
                hid0 = hidp.tile([128, 512], F16, tag="hid0")
                nc.vector.tensor_scalar(
                    out=hid0[:, :],
                    in0=sb_bT[:, 0:512],
                    scalar1=sb_abias[:, i : i + 1],
                    scalar2=0.0,
                    op0=mybir.AluOpType.add,
                    op1=mybir.AluOpType.max,
                )
                for c in range(4):
                    off = c * 128 + i
                    nc.tensor.matmul(
                        ps_t0[:, off : off + 1],
                        lhsT=hid0[:, c * 128 : (c + 1) * 128],
                        rhs=sb_w2[:, :],
                        start=True,
                        stop=True,
                    )
            for i in range(WARM):
                hid1 = hidp.tile([128, 512], F16, tag="hid1")
                nc.vector.tensor_scalar(
                    out=hid1[:, :],
                    in0=sb_bT[:, 512:1024],
                    scalar1=sb_abias[:, i : i + 1],
                    scalar2=0.0,
                    op0=mybir.AluOpType.add,
                    op1=mybir.AluOpType.max,
                )
                for c in range(4):
                    off = c * 128 + i
                    nc.tensor.matmul(
                        ps_t1[:, off : off + 1],
                        lhsT=hid1[:, c * 128 : (c + 1) * 128],
                        rhs=sb_w2[:, :],
                        start=True,
                        stop=True,
                    )

            # steady state: full-width hid rows, ~1/3 on the scalar engine
            for i in range(WARM, ROWS):
                hid = hidp.tile([128, N_CO], F16, tag="hid")
                if i % 3 == 1:
                    nc.scalar.activation(
                        out=hid[:, :],
                        in_=sb_bT[:, :],
                        func=mybir.ActivationFunctionType.Relu,
                        bias=sb_abias[:, i : i + 1],
                    )
                else:
                    nc.vector.tensor_scalar(
                        out=hid[:, :],
                        in0=sb_bT[:, :],
                        scalar1=sb_abias[:, i : i + 1],
                        scalar2=0.0,
                        op0=mybir.AluOpType.add,
                        op1=mybir.AluOpType.max,
                    )
                for c in range(8):
                    off = (c % 4) * 128 + i
                    nc.tensor.matmul(
                        ps_t[c // 4][:, off : off + 1],
                        lhsT=hid[:, c * 128 : (c + 1) * 128],
                        rhs=sb_w2[:, :],
                        start=True,
                        stop=True,
                    )

            # scoreT = relu(sT + b2); per-chunk free-dim accum -> CO_w^T cols
            # scoreT[p, c*128 + i] = score[i, c*128 + p]
            sb_scoreT = singles.tile([128, N_CO], F32)
            sb_cwT = singles.tile([128, 8], F32)
            for c in range(8):
                nc.scalar.activation(
                    out=sb_scoreT[:, c * 128 : (c + 1) * 128],
                    in_=ps_t[c // 4][:, (c % 4) * 128 : (c % 4 + 1) * 128],
                    func=mybir.ActivationFunctionType.Relu,
                    bias=sb_b2[:, :],
                    accum_out=sb_cwT[:, c : c + 1],
                )
            sb_cwT16 = singles.tile([128, 8], F16)
            nc.vector.tensor_copy(sb_cwT16[:, :], sb_cwT[:, :])

            # OP_w[i] = sum_j score[i, j]: accumulate ones-matmuls over chunks
            ps_opw = pst.tile([128, 1], F32, tag="tmp")
            for c in range(8):
                nc.tensor.matmul(
                    ps_opw[:, :],
                    lhsT=sb_scoreT[:, c * 128 : (c + 1) * 128],
                    rhs=sb_one[:, :],
                    start=(c == 0),
                    stop=(c == 7),
                )
            sb_opw = singles.tile([128, 1], F32)
            nc.vector.tensor_copy(sb_opw[:, :], ps_opw[:, :])

            # u_op | T  (T via the ones column appended to op_ext)
            ps_u = pst.tile([1, Z + 1], F32, tag="tmp")
            nc.tensor.matmul(ps_u[:, :], lhsT=sb_opw[:, :], rhs=sb_opext[:, :], start=True, stop=True)

            # u_co = sum_t cwT[:, t] . co_chunk_t
            ps_uco = pst.tile([1, Z], F32, tag="tmp")
            for t in range(8):
                nc.tensor.matmul(
                    ps_uco[:, :],
                    lhsT=sb_cwT16[:, t : t + 1],
                    rhs=sb_copk[:, t * 128 : (t + 1) * 128],
                    start=(t == 0),
                    stop=(t == 7),
                )

            sb_out = singles.tile([1, OUT_W], F32)
            nc.vector.tensor_copy(sb_out[0:1, 0 : Z + 1], ps_u[0:1, :])
            nc.vector.tensor_copy(sb_out[0:1, Z + 1 : OUT_W], ps_uco[0:1, :])
            nc.sync.dma_start(out=out[:, :], in_=sb_out[0:1, :])

    nc.compile()
    return nc


def _make_in_maps(OP_zs, CO_zs, W1, b1, W2, b2):
    op = np.asarray(OP_zs, dtype=np.float32)[0]  # [N_op, z]
    co = np.asarray(CO_zs, dtype=np.float32)[0]  # [N_co, z]
    W1 = np.asarray(W1, dtype=np.float32)
    b1 = np.asarray(b1, dtype=np.float32)
    W2 = np.asarray(W2, dtype=np.float32)
    b2 = np.asarray(b2, dtype=np.float32)

    coT = np.ascontiguousarray(co.T.astype(np.float16))  # [z, N_co]
    co_pk = np.ascontiguousarray(
        co.reshape(8, 128, Z).transpose(1, 0, 2).reshape(128, 8 * Z)
    ).astype(np.float16)  # [p, t*z] : row p holds co[t*128+p, :] for t=0..7
    vpack = np.concatenate([b1, W2, b2[:1]]).astype(np.float16)[None, :]
    shared = {
        "coT": coT,
        "co_pk": co_pk,
        "vpack": vpack,
    }
    w1b16 = W1[Z:].astype(np.float16)
    w1a16 = W1[:Z].astype(np.float16)
    in_maps = []
    for c in range(N_CORES):
        opc = op[c * ROWS : (c + 1) * ROWS]
        in_maps.append(
            {
                **shared,
                "op_ext": np.ascontiguousarray(
                    np.concatenate(
                        [opc, np.ones((ROWS, 1), dtype=np.float32)], axis=1
                    )
                ),
                "wpack": np.ascontiguousarray(
                    np.concatenate(
                        [w1b16, w1a16, opc.T.astype(np.float16)], axis=1
                    )
                ),
            }
        )
    return in_maps


def _ensure_ntff_hook():
    """This image's antenv lacks axon_hooks; synthesize it so trace=True can
    drive NTFF profiling via the axon .so (profiling-only, dev-loop)."""
    import types

    try:
        from antenv.axon_hooks import get_axon_ntff_profile_hook  # noqa: F401

        return True
    except ImportError:
        pass
    try:
        sys.path.insert(0, "/root/.axon_site")
        from trn_agent_boot.trn_boot import _ntff_profile_via_ctypes

        hook = _ntff_profile_via_ctypes("/opt/axon/libaxon_pjrt.so")
        if hook is None:
            return False
        import antenv

        mod = types.ModuleType("antenv.axon_hooks")
        _state = {"hook": hook}
        mod.set_axon_ntff_profile_hook = lambda h: _state.__setitem__("hook", h)
        mod.get_axon_ntff_profile_hook = lambda: _state["hook"]
        sys.modules["antenv.axon_hooks"] = mod
        antenv.axon_hooks = mod
        return True
    except Exception as e:  # pragma: no cover - profiling is best-effort
        print(f"ntff hook setup failed: {e}")
        return False


def kernel(OP_zs, CO_zs, W1, b1, W2, b2):
    global LAST_EXEC_NS
    if "nc" not in _CACHE:
        _CACHE["nc"] = _build()
    nc = _CACHE["nc"]
    in_maps = _make_in_maps(OP_zs, CO_zs, W1, b1, W2, b2)

    trace = bool(os.environ.get("KERNEL_PROFILE"))
    if trace:
        trace = _ensure_ntff_hook()
    res = run_bass_kernel_spmd(nc, in_maps, list(range(N_CORES)), trace=trace)
    if getattr(res, "exec_time_ns", None) is not None:
        LAST_EXEC_NS = res.exec_time_ns

    u = np.zeros(OUT_W, dtype=np.float64)
    for r in res.results:
        u += r["out"][0].astype(np.float64)
    u_op, T, u_co = u[0:Z], u[Z], u[Z + 1 :]

    if T == 0.0:
        # all-scores-zero fallback: reproduce the reference's jax.random draw
        import jax

        with jax.default_device(jax.devices("cpu")[0]):
            k = jax.random.key(1)
            OP_w = np.asarray(jax.random.uniform(k, (N_OP,)), dtype=np.float64)
            CO_w = np.asarray(
                jax.random.uniform(jax.random.fold_in(k, 1), (N_CO,)),
                dtype=np.float64,
            )
        op = np.asarray(OP_zs, dtype=np.float64)[0]
        co = np.asarray(CO_zs, dtype=np.float64)[0]
        u_op, u_co = OP_w @ op, CO_w @ co
        return (
            (u_op / OP_w.sum())[None].astype(np.float32),
            (u_co / CO_w.sum())[None].astype(np.float32),
        )

    return (
        (u_op / T)[None].astype(np.float32),
        (u_co / T)[None].astype(np.float32),
    )


# revision 20
# speedup vs baseline: 1.0068x; 1.0068x over previous
"""Trainium2 Bass kernel for nn_FFN_pairwise_z (pairwise-concat FFN scoring).

Math (see reference):
    a = op @ W1[:z]           [N_op, h]
    b = co @ W1[z:]           [N_co, h]
    score_ij = relu( relu(a_i + b_j + b1) . W2 + b2 )
    OP_w[i] = sum_j score, CO_w[j] = sum_i score, T = sum_ij score
    out = (OP_w @ op / T,  CO_w @ co / T)       two [1, z] vectors

Sharding: N_op rows split across 8 cores (128 rows each).  Each core
computes its score block [128, 1024] without materializing it in DRAM and
emits only partial sums:
    u_op_part   = OP_w_local @ op_local        [z]
    T_part      = sum(OP_w_local)              [1]
    u_co_part   = CO_w_part @ co               [z]
packed as one [1, 2z+1] output.  The host adds the 8 partials and divides
by T (the "all-reduce + normalize" step of the hinted strategy, done on
host since it is 257 floats).

Device pipeline per core (layout: h on partitions):
    bT   = (co @ W1b)^T     [h=128, N_co]   fp16, via 2 fp32 matmuls
    abias= (op_l @ W1a)^T + b1  [h, 128]    fp32
    per i in 0..127:
        hid_i = max(bT + abias[:, i], 0)    one DVE tensor_scalar (fp16, 4x)
        s[i, :] = W2^T @ hid_i              two fp16 matmuls -> PSUM row i
    score = relu(s + b2) (ACT, accum_out gives OP_w_local for free)
    u_op|T  : one matmul  lhsT=OP_w_local, rhs=[op_l | ones]
    CO_w^T  : 8 matmuls   lhsT=score chunk, rhs=ones
    u_co    : 8 accumulating matmuls lhsT=CO_w^T col, rhs=co chunk
"""

import os
import sys

for _p in ("/opt/trn_rl_repo", "/root/.axon_site/_ro/trn_rl_repo"):
    if os.path.isdir(_p) and _p not in sys.path:
        sys.path.insert(0, _p)

import numpy as np

import concourse.bacc as bacc
import concourse.tile as tile
from concourse import mybir
from concourse.bass_utils import run_bass_kernel_spmd

N_OP, N_CO, Z, H = 1024, 1024, 128, 128
N_CORES = 8
ROWS = N_OP // N_CORES  # 128 op-rows per core
F32 = mybir.dt.float32
F16 = mybir.dt.float16
OUT_W = 2 * Z + 1  # u_op (z) | T (1) | u_co (z)

_CACHE = {}
LAST_EXEC_NS = None


def _build():
    nc = bacc.Bacc("TRN2", target_bir_lowering=False, debug=False)

    op_ext = nc.dram_tensor("op_ext", [ROWS, Z + 1], F32, kind="ExternalInput")
    coT = nc.dram_tensor("coT", [Z, N_CO], F16, kind="ExternalInput")
    co_pk = nc.dram_tensor("co_pk", [128, N_CO], F16, kind="ExternalInput")
    # w1b | w1a | op_lT packed as one fp16 tensor (single DMA)
    wpack = nc.dram_tensor("wpack", [Z, 2 * H + ROWS], F16, kind="ExternalInput")
    # single row: [b1 (128) | W2 (128) | b2 (1)] fp16 (single DMA)
    vpack = nc.dram_tensor("vpack", [1, 2 * H + 1], F16, kind="ExternalInput")
    out = nc.dram_tensor("out", [1, OUT_W], F32, kind="ExternalOutput")

    with tile.TileContext(nc) as tc:
        with (
            tc.tile_pool(name="singles", bufs=1) as singles,
            tc.tile_pool(name="hidp", bufs=4) as hidp,
            tc.tile_pool(name="ps_main", bufs=1, space="PSUM") as psm,
            tc.tile_pool(name="ps_tmp", bufs=2, space="PSUM") as pst,
        ):
            # 5 input DMAs total, issue spread across three engines so the
            # per-dma_start descriptor-gen cost (~0.6us) does not serialize.
            sb_coT = singles.tile([128, N_CO], F16)
            nc.sync.dma_start(out=sb_coT[:, 0:512], in_=coT[:, 0:512])
            nc.scalar.dma_start(out=sb_coT[:, 512:1024], in_=coT[:, 512:1024])
            sb_wpack = singles.tile([128, 2 * H + ROWS], F16)
            nc.gpsimd.dma_start(out=sb_wpack[:, :], in_=wpack[:, :])
            sb_w1b = sb_wpack[:, 0:H]
            sb_w1a = sb_wpack[:, H : 2 * H]
            sb_oplT = sb_wpack[:, 2 * H : 2 * H + ROWS]
            sb_vpack = singles.tile([1, 2 * H + 1], F16)
            nc.scalar.dma_start(out=sb_vpack[0:1, :], in_=vpack[0:1, :])
            sb_b1r = sb_vpack[0:1, 0:H]
            sb_w2r = sb_vpack[0:1, H : 2 * H]
            sb_b2cell = sb_vpack[0:1, 2 * H : 2 * H + 1]
            # late-needed loads (tail only)
            sb_copk = singles.tile([128, N_CO], F16)
            nc.gpsimd.dma_start(out=sb_copk[:, :], in_=co_pk[:, :])
            sb_opext = singles.tile([128, Z + 1], F32)
            nc.scalar.dma_start(out=sb_opext[:, :], in_=op_ext[:, :])

            # on-chip constants / broadcasts (no partition-scattered DMAs)
            sb_onesrow = singles.tile([1, ROWS], F16)
            nc.vector.memset(sb_onesrow[0:1, :], 1.0)
            sb_ident = singles.tile([1, 1], F16)
            nc.vector.memset(sb_ident[0:1, :], 1.0)
            sb_one = singles.tile([128, 1], F32)
            nc.vector.memset(sb_one[:, :], 1.0)

            # bT[h, j] = sum_z W1b[z, h] * coT[z, j], stored fp16
            sb_bT = singles.tile([128, N_CO], F16)
            for half in range(2):
                ps_b = pst.tile([128, 512], F32, tag="tmp")
                nc.tensor.matmul(
                    ps_b[:, :],
                    lhsT=sb_w1b[:, :],
                    rhs=sb_coT[:, half * 512 : (half + 1) * 512],
                    start=True,
                    stop=True,
                )
                if half == 0:
                    nc.scalar.copy(sb_bT[:, 0:512], ps_b)
                else:
                    nc.vector.tensor_copy(sb_bT[:, 512:1024], ps_b[:, :])

            # w2 column via PE transpose of the [1,128] row
            ps_w2 = pst.tile([128, 1], F16, tag="tmp")
            nc.tensor.transpose(ps_w2[:, :], sb_w2r[0:1, :], sb_ident[0:1, :])
            sb_w2 = singles.tile([128, 1], F16)
            nc.vector.tensor_copy(sb_w2[:, :], ps_w2[:, :])

            # b2 column: [128,1] broadcast of the scalar via K=1 matmul
            ps_b2 = pst.tile([128, 1], F32, tag="tmp")
            nc.tensor.matmul(ps_b2[:, :], lhsT=sb_onesrow[0:1, :], rhs=sb_b2cell[0:1, :], start=True, stop=True)
            sb_b2 = singles.tile([128, 1], F32)
            nc.vector.tensor_copy(sb_b2[:, :], ps_b2[:, :])

            # abias[h, i] = sum_z W1a[z,h] opT[z,i] + b1[h] (b1 folded via K=1
            # accumulate matmul: lhsT=b1row, rhs=ones_row)
            ps_a = pst.tile([128, ROWS], F32, tag="tmp")
            nc.tensor.matmul(ps_a[:, :], lhsT=sb_w1a[:, :], rhs=sb_oplT[:, :], start=True, stop=False)
            nc.tensor.matmul(ps_a[:, :], lhsT=sb_b1r[0:1, :], rhs=sb_onesrow[0:1, :], start=False, stop=True)
            sb_abias = singles.tile([128, ROWS], F32)
            nc.vector.tensor_copy(sb_abias[:, :], ps_a[:, :])

            # main pairwise loop.  hid chunks go through the PE as the
            # STATIONARY operand (fp16 weight loads stream 2 elem/cycle),
            # W2 as the moving operand (N=1): one [128,1] psum column per
            # (i, j-chunk), written at free offset (c%4)*128 + i.
            # ps_t0 holds j-chunks 0..3, ps_t1 chunks 4..7; layout [j, (c, i)].
            ps_t0 = psm.tile([128, 512], F32, tag="s0")
            ps_t1 = psm.tile([128, 512], F32, tag="s1")
            ps_t = (ps_t0, ps_t1)
            CORD = (0, 4, 1, 5, 2, 6, 3, 7)  # alternate PSUM banks
            for i in range(ROWS):
                hid = hidp.tile([128, N_CO], F16, tag="hid")
                if i % 3 == 2:
                    nc.scalar.activation(
                        out=hid[:, :],
                        in_=sb_bT[:, :],
                        func=mybir.ActivationFunctionType.Relu,
                        bias=sb_abias[:, i : i + 1],
                    )
                else:
                    nc.vector.tensor_scalar(
                        out=hid[:, :],
                        in0=sb_bT[:, :],
                        scalar1=sb_abias[:, i : i + 1],
                        scalar2=0.0,
                        op0=mybir.AluOpType.add,
                        op1=mybir.AluOpType.max,
                    )
                for c in CORD:
                    off = (c % 4) * 128 + i
                    nc.tensor.matmul(
                        ps_t[c // 4][:, off : off + 1],
                        lhsT=hid[:, c * 128 : (c + 1) * 128],
                        rhs=sb_w2[:, :],
                        start=True,
                        stop=True,
                    )

            # scoreT = relu(sT + b2); per-chunk free-dim accum -> CO_w^T cols
            # scoreT[p, c*128 + i] = score[i, c*128 + p]
            sb_scoreT = singles.tile([128, N_CO], F32)
            sb_cwT = singles.tile([128, 8], F32)
            for c in range(8):
                nc.scalar.activation(
                    out=sb_scoreT[:, c * 128 : (c + 1) * 128],
                    in_=ps_t[c // 4][:, (c % 4) * 128 : (c % 4 + 1) * 128],
                    func=mybir.ActivationFunctionType.Relu,
                    bias=sb_b2[:, :],
                    accum_out=sb_cwT[:, c : c + 1],
                )
            sb_cwT16 = singles.tile([128, 8], F16)
            nc.vector.tensor_copy(sb_cwT16[:, :], sb_cwT[:, :])

            # OP_w[i] = sum_j score[i, j]: accumulate ones-matmuls over chunks
            ps_opw = pst.tile([128, 1], F32, tag="tmp")
            for c in range(8):
                nc.tensor.matmul(
                    ps_opw[:, :],
                    lhsT=sb_scoreT[:, c * 128 : (c + 1) * 128],
                    rhs=sb_one[:, :],
                    start=(c == 0),
                    stop=(c == 7),
                )
            sb_opw = singles.tile([128, 1], F32)
            nc.vector.tensor_copy(sb_opw[:, :], ps_opw[:, :])

            # u_op | T  (T via the ones column appended to op_ext)
            ps_u = pst.tile([1, Z + 1], F32, tag="tmp")
            nc.tensor.matmul(ps_u[:, :], lhsT=sb_opw[:, :], rhs=sb_opext[:, :], start=True, stop=True)

            # u_co = sum_t cwT[:, t] . co_chunk_t
            ps_uco = pst.tile([1, Z], F32, tag="tmp")
            for t in range(8):
                nc.tensor.matmul(
                    ps_uco[:, :],
                    lhsT=sb_cwT16[:, t : t + 1],
                    rhs=sb_copk[:, t * 128 : (t + 1) * 128],
                    start=(t == 0),
                    stop=(t == 7),
                )

            sb_out = singles.tile([1, OUT_W], F32)
            nc.vector.tensor_copy(sb_out[0:1, 0 : Z + 1], ps_u[0:1, :])
            nc.vector.tensor_copy(sb_out[0:1, Z + 1 : OUT_W], ps_uco[0:1, :])
            nc.sync.dma_start(out=out[:, :], in_=sb_out[0:1, :])

    nc.compile()
    return nc


def _make_in_maps(OP_zs, CO_zs, W1, b1, W2, b2):
    op = np.asarray(OP_zs, dtype=np.float32)[0]  # [N_op, z]
    co = np.asarray(CO_zs, dtype=np.float32)[0]  # [N_co, z]
    W1 = np.asarray(W1, dtype=np.float32)
    b1 = np.asarray(b1, dtype=np.float32)
    W2 = np.asarray(W2, dtype=np.float32)
    b2 = np.asarray(b2, dtype=np.float32)

    coT = np.ascontiguousarray(co.T.astype(np.float16))  # [z, N_co]
    co_pk = np.ascontiguousarray(
        co.reshape(8, 128, Z).transpose(1, 0, 2).reshape(128, 8 * Z)
    ).astype(np.float16)  # [p, t*z] : row p holds co[t*128+p, :] for t=0..7
    vpack = np.concatenate([b1, W2, b2[:1]]).astype(np.float16)[None, :]
    shared = {
        "coT": coT,
        "co_pk": co_pk,
        "vpack": vpack,
    }
    w1b16 = W1[Z:].astype(np.float16)
    w1a16 = W1[:Z].astype(np.float16)
    in_maps = []
    for c in range(N_CORES):
        opc = op[c * ROWS : (c + 1) * ROWS]
        in_maps.append(
            {
                **shared,
                "op_ext": np.ascontiguousarray(
                    np.concatenate(
                        [opc, np.ones((ROWS, 1), dtype=np.float32)], axis=1
                    )
                ),
                "wpack": np.ascontiguousarray(
                    np.concatenate(
                        [w1b16, w1a16, opc.T.astype(np.float16)], axis=1
                    )
                ),
            }
        )
    return in_maps


def _ensure_ntff_hook():
    """This image's antenv lacks axon_hooks; synthesize it so trace=True can
    drive NTFF profiling via the axon .so (profiling-only, dev-loop)."""
    import types

    try:
        from antenv.axon_hooks import get_axon_ntff_profile_hook  # noqa: F401

        return True
    except ImportError:
        pass
    try:
        sys.path.insert(0, "/root/.axon_site")
        from trn_agent_boot.trn_boot import _ntff_profile_via_ctypes

        hook = _ntff_profile_via_ctypes("/opt/axon/libaxon_pjrt.so")
        if hook is None:
            return False
        import antenv

        mod = types.ModuleType("antenv.axon_hooks")
        _state = {"hook": hook}
        mod.set_axon_ntff_profile_hook = lambda h: _state.__setitem__("hook", h)
        mod.get_axon_ntff_profile_hook = lambda: _state["hook"]
        sys.modules["antenv.axon_hooks"] = mod
        antenv.axon_hooks = mod
        return True
    except Exception as e:  # pragma: no cover - profiling is best-effort
        print(f"ntff hook setup failed: {e}")
        return False


def kernel(OP_zs, CO_zs, W1, b1, W2, b2):
    global LAST_EXEC_NS
    if "nc" not in _CACHE:
        _CACHE["nc"] = _build()
    nc = _CACHE["nc"]
    in_maps = _make_in_maps(OP_zs, CO_zs, W1, b1, W2, b2)

    trace = bool(os.environ.get("KERNEL_PROFILE"))
    if trace:
        trace = _ensure_ntff_hook()
    res = run_bass_kernel_spmd(nc, in_maps, list(range(N_CORES)), trace=trace)
    if getattr(res, "exec_time_ns", None) is not None:
        LAST_EXEC_NS = res.exec_time_ns

    u = np.zeros(OUT_W, dtype=np.float64)
    for r in res.results:
        u += r["out"][0].astype(np.float64)
    u_op, T, u_co = u[0:Z], u[Z], u[Z + 1 :]

    if T == 0.0:
        # all-scores-zero fallback: reproduce the reference's jax.random draw
        import jax

        with jax.default_device(jax.devices("cpu")[0]):
            k = jax.random.key(1)
            OP_w = np.asarray(jax.random.uniform(k, (N_OP,)), dtype=np.float64)
            CO_w = np.asarray(
                jax.random.uniform(jax.random.fold_in(k, 1), (N_CO,)),
                dtype=np.float64,
            )
        op = np.asarray(OP_zs, dtype=np.float64)[0]
        co = np.asarray(CO_zs, dtype=np.float64)[0]
        u_op, u_co = OP_w @ op, CO_w @ co
        return (
            (u_op / OP_w.sum())[None].astype(np.float32),
            (u_co / CO_w.sum())[None].astype(np.float32),
        )

    return (
        (u_op / T)[None].astype(np.float32),
        (u_co / T)[None].astype(np.float32),
    )


# revision 21
# speedup vs baseline: 1.1230x; 1.1154x over previous
"""Trainium2 Bass kernel for nn_FFN_pairwise_z (pairwise-concat FFN scoring).

Math (see reference):
    a = op @ W1[:z]           [N_op, h]
    b = co @ W1[z:]           [N_co, h]
    score_ij = relu( relu(a_i + b_j + b1) . W2 + b2 )
    OP_w[i] = sum_j score, CO_w[j] = sum_i score, T = sum_ij score
    out = (OP_w @ op / T,  CO_w @ co / T)       two [1, z] vectors

Sharding: N_op rows split across 8 cores (128 rows each).  Each core
computes its score block [128, 1024] without materializing it in DRAM and
emits only partial sums:
    u_op_part   = OP_w_local @ op_local        [z]
    T_part      = sum(OP_w_local)              [1]
    u_co_part   = CO_w_part @ co               [z]
packed as one [1, 2z+1] output.  The host adds the 8 partials and divides
by T (the "all-reduce + normalize" step of the hinted strategy, done on
host since it is 257 floats).

Device pipeline per core (layout: h on partitions):
    bT   = (co @ W1b)^T     [h=128, N_co]   fp16, via 2 fp32 matmuls
    abias= (op_l @ W1a)^T + b1  [h, 128]    fp32
    per i in 0..127:
        hid_i = max(bT + abias[:, i], 0)    one DVE tensor_scalar (fp16, 4x)
        s[i, :] = W2^T @ hid_i              two fp16 matmuls -> PSUM row i
    score = relu(s + b2) (ACT, accum_out gives OP_w_local for free)
    u_op|T  : one matmul  lhsT=OP_w_local, rhs=[op_l | ones]
    CO_w^T  : 8 matmuls   lhsT=score chunk, rhs=ones
    u_co    : 8 accumulating matmuls lhsT=CO_w^T col, rhs=co chunk
"""

import os
import sys

for _p in ("/opt/trn_rl_repo", "/root/.axon_site/_ro/trn_rl_repo"):
    if os.path.isdir(_p) and _p not in sys.path:
        sys.path.insert(0, _p)

import numpy as np

import concourse.bacc as bacc
import concourse.tile as tile
from concourse import mybir
from concourse.bass_utils import run_bass_kernel_spmd

N_OP, N_CO, Z, H = 1024, 1024, 128, 128
N_CORES = 8
ROWS = N_OP // N_CORES  # 128 op-rows per core
F32 = mybir.dt.float32
F16 = mybir.dt.float16
OUT_W = 2 * Z + 1  # u_op (z) | T (1) | u_co (z)

_CACHE = {}
LAST_EXEC_NS = None


def _build():
    nc = bacc.Bacc("TRN2", target_bir_lowering=False, debug=False)

    op_ext = nc.dram_tensor("op_ext", [ROWS, Z + 1], F32, kind="ExternalInput")
    coT = nc.dram_tensor("coT", [Z, N_CO], F16, kind="ExternalInput")
    co_pk = nc.dram_tensor("co_pk", [128, N_CO], F16, kind="ExternalInput")
    # w1b | w1a | op_lT packed as one fp16 tensor (single DMA)
    wpack = nc.dram_tensor("wpack", [Z, 2 * H + ROWS], F16, kind="ExternalInput")
    # single row: [b1 (128) | W2 (128) | b2 (1)] fp16 (single DMA)
    vpack = nc.dram_tensor("vpack", [1, 2 * H + 1], F16, kind="ExternalInput")
    out = nc.dram_tensor("out", [1, OUT_W], F32, kind="ExternalOutput")

    with tile.TileContext(nc) as tc:
        with (
            tc.tile_pool(name="singles", bufs=1) as singles,
            tc.tile_pool(name="hidp", bufs=6) as hidp,
            tc.tile_pool(name="ps_main", bufs=1, space="PSUM") as psm,
            tc.tile_pool(name="ps_tmp", bufs=2, space="PSUM") as pst,
        ):
            # 5 input DMAs total, issue spread across three engines so the
            # per-dma_start descriptor-gen cost (~0.6us) does not serialize.
            sb_coT = singles.tile([128, N_CO], F16)
            nc.sync.dma_start(out=sb_coT[:, 0:512], in_=coT[:, 0:512])
            nc.scalar.dma_start(out=sb_coT[:, 512:1024], in_=coT[:, 512:1024])
            sb_wpack = singles.tile([128, 2 * H + ROWS], F16)
            nc.gpsimd.dma_start(out=sb_wpack[:, :], in_=wpack[:, :])
            sb_w1b = sb_wpack[:, 0:H]
            sb_w1a = sb_wpack[:, H : 2 * H]
            sb_oplT = sb_wpack[:, 2 * H : 2 * H + ROWS]
            sb_vpack = singles.tile([1, 2 * H + 1], F16)
            nc.scalar.dma_start(out=sb_vpack[0:1, :], in_=vpack[0:1, :])
            sb_b1r = sb_vpack[0:1, 0:H]
            sb_w2r = sb_vpack[0:1, H : 2 * H]
            sb_b2cell = sb_vpack[0:1, 2 * H : 2 * H + 1]
            # late-needed loads (tail only)
            sb_copk = singles.tile([128, N_CO], F16)
            nc.gpsimd.dma_start(out=sb_copk[:, :], in_=co_pk[:, :])
            sb_opext = singles.tile([128, Z + 1], F32)
            nc.gpsimd.dma_start(out=sb_opext[:, :], in_=op_ext[:, :])

            # on-chip constants / broadcasts (no partition-scattered DMAs)
            sb_onesrow = singles.tile([1, ROWS], F16)
            nc.vector.memset(sb_onesrow[0:1, :], 1.0)
            sb_ident = singles.tile([1, 1], F16)
            nc.vector.memset(sb_ident[0:1, :], 1.0)
            sb_one = singles.tile([128, 1], F32)
            nc.vector.memset(sb_one[:, :], 1.0)

            # w2 column via PE transpose of the [1,128] row
            ps_w2 = pst.tile([128, 1], F16, tag="tmp")
            nc.tensor.transpose(ps_w2[:, :], sb_w2r[0:1, :], sb_ident[0:1, :])
            sb_w2 = singles.tile([128, 1], F16)
            nc.vector.tensor_copy(sb_w2[:, :], ps_w2[:, :])

            # b2 column: [128,1] broadcast of the scalar via K=1 matmul
            ps_b2 = pst.tile([128, 1], F32, tag="tmp")
            nc.tensor.matmul(ps_b2[:, :], lhsT=sb_onesrow[0:1, :], rhs=sb_b2cell[0:1, :], start=True, stop=True)
            sb_b2 = singles.tile([128, 1], F32)
            nc.vector.tensor_copy(sb_b2[:, :], ps_b2[:, :])

            # abias[h, i] = sum_z W1a[z,h] opT[z,i] + b1[h] (b1 folded via K=1
            # accumulate matmul: lhsT=b1row, rhs=ones_row)
            ps_a = pst.tile([128, ROWS], F32, tag="tmp")
            nc.tensor.matmul(ps_a[:, :], lhsT=sb_w1a[:, :], rhs=sb_oplT[:, :], start=True, stop=False)
            nc.tensor.matmul(ps_a[:, :], lhsT=sb_b1r[0:1, :], rhs=sb_onesrow[0:1, :], start=False, stop=True)
            sb_abias = singles.tile([128, ROWS], F32)
            nc.vector.tensor_copy(sb_abias[:, :], ps_a[:, :])

            # bT[h, j] = sum_z W1b[z, h] * coT[z, j], stored fp16
            sb_bT = singles.tile([128, N_CO], F16)
            for half in range(2):
                ps_b = pst.tile([128, 512], F32, tag="tmp")
                nc.tensor.matmul(
                    ps_b[:, :],
                    lhsT=sb_w1b[:, :],
                    rhs=sb_coT[:, half * 512 : (half + 1) * 512],
                    start=True,
                    stop=True,
                )
                if half == 0:
                    nc.scalar.copy(sb_bT[:, 0:512], ps_b)
                else:
                    nc.vector.tensor_copy(sb_bT[:, 512:1024], ps_b[:, :])

            # main pairwise loop.  hid chunks go through the PE as the
            # STATIONARY operand (fp16 weight loads stream 2 elem/cycle),
            # W2 as the moving operand (N=1): one [128,1] psum column per
            # (i, j-chunk), written at free offset (c%4)*128 + i.
            # ps_t0 holds j-chunks 0..3, ps_t1 chunks 4..7; layout [j, (c, i)].
            ps_t0 = psm.tile([128, 512], F32, tag="s0")
            ps_t1 = psm.tile([128, 512], F32, tag="s1")
            ps_t = (ps_t0, ps_t1)
            CORD = (0, 4, 1, 5, 2, 6, 3, 7)  # alternate PSUM banks
            for i in range(ROWS):
                hid = hidp.tile([128, N_CO], F16, tag="hid")
                if i % 4 == 3:
                    nc.scalar.activation(
                        out=hid[:, :],
                        in_=sb_bT[:, :],
                        func=mybir.ActivationFunctionType.Relu,
                        bias=sb_abias[:, i : i + 1],
                    )
                else:
                    nc.vector.tensor_scalar(
                        out=hid[:, :],
                        in0=sb_bT[:, :],
                        scalar1=sb_abias[:, i : i + 1],
                        scalar2=0.0,
                        op0=mybir.AluOpType.add,
                        op1=mybir.AluOpType.max,
                    )
                for c in CORD:
                    off = (c % 4) * 128 + i
                    nc.tensor.matmul(
                        ps_t[c // 4][:, off : off + 1],
                        lhsT=hid[:, c * 128 : (c + 1) * 128],
                        rhs=sb_w2[:, :],
                        start=True,
                        stop=True,
                    )

            # scoreT = relu(sT + b2); per-chunk free-dim accum -> CO_w^T cols
            # scoreT[p, c*128 + i] = score[i, c*128 + p]
            sb_scoreT = singles.tile([128, N_CO], F32)
            sb_cwT = singles.tile([128, 8], F32)
            for c in range(8):
                nc.scalar.activation(
                    out=sb_scoreT[:, c * 128 : (c + 1) * 128],
                    in_=ps_t[c // 4][:, (c % 4) * 128 : (c % 4 + 1) * 128],
                    func=mybir.ActivationFunctionType.Relu,
                    bias=sb_b2[:, :],
                    accum_out=sb_cwT[:, c : c + 1],
                )
            sb_cwT16 = singles.tile([128, 8], F16)
            nc.vector.tensor_copy(sb_cwT16[:, :], sb_cwT[:, :])

            # OP_w[i] = sum_j score[i, j]: accumulate ones-matmuls over chunks
            ps_opw = pst.tile([128, 1], F32, tag="tmp")
            for c in range(8):
                nc.tensor.matmul(
                    ps_opw[:, :],
                    lhsT=sb_scoreT[:, c * 128 : (c + 1) * 128],
                    rhs=sb_one[:, :],
                    start=(c == 0),
                    stop=(c == 7),
                )
            sb_opw = singles.tile([128, 1], F32)
            nc.vector.tensor_copy(sb_opw[:, :], ps_opw[:, :])

            # u_op | T  (T via the ones column appended to op_ext)
            ps_u = pst.tile([1, Z + 1], F32, tag="tmp")
            nc.tensor.matmul(ps_u[:, :], lhsT=sb_opw[:, :], rhs=sb_opext[:, :], start=True, stop=True)

            # u_co = sum_t cwT[:, t] . co_chunk_t
            ps_uco = pst.tile([1, Z], F32, tag="tmp")
            for t in range(8):
                nc.tensor.matmul(
                    ps_uco[:, :],
                    lhsT=sb_cwT16[:, t : t + 1],
                    rhs=sb_copk[:, t * 128 : (t + 1) * 128],
                    start=(t == 0),
                    stop=(t == 7),
                )

            sb_out = singles.tile([1, OUT_W], F32)
            nc.vector.tensor_copy(sb_out[0:1, 0 : Z + 1], ps_u[0:1, :])
            nc.vector.tensor_copy(sb_out[0:1, Z + 1 : OUT_W], ps_uco[0:1, :])
            nc.sync.dma_start(out=out[:, :], in_=sb_out[0:1, :])

    nc.compile()
    return nc


def _make_in_maps(OP_zs, CO_zs, W1, b1, W2, b2):
    op = np.asarray(OP_zs, dtype=np.float32)[0]  # [N_op, z]
    co = np.asarray(CO_zs, dtype=np.float32)[0]  # [N_co, z]
    W1 = np.asarray(W1, dtype=np.float32)
    b1 = np.asarray(b1, dtype=np.float32)
    W2 = np.asarray(W2, dtype=np.float32)
    b2 = np.asarray(b2, dtype=np.float32)

    coT = np.ascontiguousarray(co.T.astype(np.float16))  # [z, N_co]
    co_pk = np.ascontiguousarray(
        co.reshape(8, 128, Z).transpose(1, 0, 2).reshape(128, 8 * Z)
    ).astype(np.float16)  # [p, t*z] : row p holds co[t*128+p, :] for t=0..7
    vpack = np.concatenate([b1, W2, b2[:1]]).astype(np.float16)[None, :]
    shared = {
        "coT": coT,
        "co_pk": co_pk,
        "vpack": vpack,
    }
    w1b16 = W1[Z:].astype(np.float16)
    w1a16 = W1[:Z].astype(np.float16)
    in_maps = []
    for c in range(N_CORES):
        opc = op[c * ROWS : (c + 1) * ROWS]
        in_maps.append(
            {
                **shared,
                "op_ext": np.ascontiguousarray(
                    np.concatenate(
                        [opc, np.ones((ROWS, 1), dtype=np.float32)], axis=1
                    )
                ),
                "wpack": np.ascontiguousarray(
                    np.concatenate(
                        [w1b16, w1a16, opc.T.astype(np.float16)], axis=1
                    )
                ),
            }
        )
    return in_maps


def _ensure_ntff_hook():
    """This image's antenv lacks axon_hooks; synthesize it so trace=True can
    drive NTFF profiling via the axon .so (profiling-only, dev-loop)."""
    import types

    try:
        from antenv.axon_hooks import get_axon_ntff_profile_hook  # noqa: F401

        return True
    except ImportError:
        pass
    try:
        sys.path.insert(0, "/root/.axon_site")
        from trn_agent_boot.trn_boot import _ntff_profile_via_ctypes

        hook = _ntff_profile_via_ctypes("/opt/axon/libaxon_pjrt.so")
        if hook is None:
            return False
        import antenv

        mod = types.ModuleType("antenv.axon_hooks")
        _state = {"hook": hook}
        mod.set_axon_ntff_profile_hook = lambda h: _state.__setitem__("hook", h)
        mod.get_axon_ntff_profile_hook = lambda: _state["hook"]
        sys.modules["antenv.axon_hooks"] = mod
        antenv.axon_hooks = mod
        return True
    except Exception as e:  # pragma: no cover - profiling is best-effort
        print(f"ntff hook setup failed: {e}")
        return False


def kernel(OP_zs, CO_zs, W1, b1, W2, b2):
    global LAST_EXEC_NS
    if "nc" not in _CACHE:
        _CACHE["nc"] = _build()
    nc = _CACHE["nc"]
    in_maps = _make_in_maps(OP_zs, CO_zs, W1, b1, W2, b2)

    trace = bool(os.environ.get("KERNEL_PROFILE"))
    if trace:
        trace = _ensure_ntff_hook()
    res = run_bass_kernel_spmd(nc, in_maps, list(range(N_CORES)), trace=trace)
    if getattr(res, "exec_time_ns", None) is not None:
        LAST_EXEC_NS = res.exec_time_ns

    u = np.zeros(OUT_W, dtype=np.float64)
    for r in res.results:
        u += r["out"][0].astype(np.float64)
    u_op, T, u_co = u[0:Z], u[Z], u[Z + 1 :]

    if T == 0.0:
        # all-scores-zero fallback: reproduce the reference's jax.random draw
        import jax

        with jax.default_device(jax.devices("cpu")[0]):
            k = jax.random.key(1)
            OP_w = np.asarray(jax.random.uniform(k, (N_OP,)), dtype=np.float64)
            CO_w = np.asarray(
                jax.random.uniform(jax.random.fold_in(k, 1), (N_CO,)),
                dtype=np.float64,
            )
        op = np.asarray(OP_zs, dtype=np.float64)[0]
        co = np.asarray(CO_zs, dtype=np.float64)[0]
        u_op, u_co = OP_w @ op, CO_w @ co
        return (
            (u_op / OP_w.sum())[None].astype(np.float32),
            (u_co / CO_w.sum())[None].astype(np.float32),
        )

    return (
        (u_op / T)[None].astype(np.float32),
        (u_co / T)[None].astype(np.float32),
    )
